# revision 1
# baseline (speedup 1.0000x reference)
"""nn_DenseGeneral: AQT-style int8 fake-quant einsum 'btd,dh->bth' on 8 NeuronCores.

Math: fake-quant values are integers in [-127,127] -> exact in bf16; the
integer products accumulate exactly in fp32 PSUM. Key fold: the kernel-side
dequant scale sk is folded into the matmul operand itself:
    wq = bf16(round(k / sk) * sk)        [bf16 rounding adds ~2e-3 rel err,
                                          well under the 2e-2 tolerance]
so  out = (qi @ wq) * si  with a single per-row (per-partition) scale in the
epilogue.

Two SPMD launches over 8 cores:
  A (kernel dequant): core c loads kernel column-slice [:, 512c:512(c+1)]
     (f32, 2 MB), computes per-column absmax via an on-chip tree +
     gpsimd all-reduce, and writes the dequantized bf16 slice (1 MB).
  B (fused quantize+matmul): row-parallel. Core c loads its raw input rows
     [1024, 1024] f32 and the full wq (host-assembled, replicated, 8 MB
     bf16). Quantizes rows on-chip (per-row scale si stays in SBUF),
     transposes the quantized bf16 via PE (interleaved with the matmul
     stream), runs 512 bf16 matmuls accumulating in 1-bank PSUM tiles, and
     scales by si in the epilogue (per-partition scalar on DVE).
"""
import sys

if "/opt/trn_rl_repo" not in sys.path:
    sys.path.insert(0, "/opt/trn_rl_repo")

import numpy as np
import ml_dtypes

import concourse.bacc as bacc
import concourse.mybir as mybir
import concourse.tile as tile
from concourse import bass_isa
from concourse.masks import make_identity
from concourse.bass2jax import (
    _bass_exec_p,
    install_neuronx_cc_hook,
    partition_id_tensor,
)

f32 = mybir.dt.float32
bf16 = mybir.dt.bfloat16
A_ = mybir.AluOpType
AX = mybir.AxisListType
AF = mybir.ActivationFunctionType

MAGIC = float(np.float32(1.5 * 2**23))   # fp32 round-to-int magic
C127 = float(np.float32(1.0 / 127.0))
EPS = 1e-8

NCORES = 8
B, T, D, H = 4, 2048, 1024, 4096
BT = B * T                 # 8192 rows total
TR = BT // NCORES          # 1024 rows per core
HS = H // NCORES           # 512 kernel cols per core
DCH = D // 128             # 8 contraction chunks
TT = TR // 128             # 8 T-tiles per core
NHH = H // 2048            # 2 wq-load halves


def _build_prog_a2(loop_n=None):
    """Launch A: dequantized-quantized kernel column slice (scales folded).

    loop_n: when set, wrap the whole body in a hardware For_i loop that
    executes it loop_n times -- used only for device-time measurement.
    """
    nc = bacc.Bacc("TRN2", target_bir_lowering=False, debug=False)
    k_dram = nc.dram_tensor("ka", [D, HS], f32, kind="ExternalInput")
    wq_o = nc.dram_tensor("wq", [D, HS], bf16, kind="ExternalOutput")

    with tile.TileContext(nc) as tc:
        import contextlib
        with (
            tc.tile_pool(name="kp", bufs=2) as kp,
            tc.tile_pool(name="sb", bufs=3) as sb,
            (tc.For_i(0, loop_n, 1) if loop_n else contextlib.nullcontext()),
        ):
            k_sb = kp.tile([128, DCH, HS], f32)
            for c2 in range(DCH // 2):
                ring = nc.sync if c2 % 2 == 0 else nc.scalar
                ring.dma_start(
                    k_sb[:, 2 * c2:2 * c2 + 2, :],
                    k_dram[c2 * 256:(c2 + 1) * 256, :].rearrange(
                        "(c p) h -> p c h", c=2))
            # abs-max over chunks: two half reduces pipelined with the
            # loads, then a combine and a gpsimd partition all-reduce
            cm1 = kp.tile([128, HS], f32)
            nc.vector.tensor_reduce(cm1[:],
                                    k_sb[:, 0:4, :].rearrange("p c h -> p h c"),
                                    axis=AX.X, op=A_.max,
                                    apply_absolute_value=True)
            cm2 = kp.tile([128, HS], f32)
            nc.vector.tensor_reduce(cm2[:],
                                    k_sb[:, 4:8, :].rearrange("p c h -> p h c"),
                                    axis=AX.X, op=A_.max,
                                    apply_absolute_value=True)
            cm = kp.tile([128, HS], f32)
            nc.vector.tensor_tensor(out=cm[:], in0=cm1[:], in1=cm2[:],
                                    op=A_.max)
            colmax = kp.tile([128, HS], f32)
            nc.gpsimd.partition_all_reduce(colmax[:], cm[:], channels=128,
                                           reduce_op=bass_isa.ReduceOp.max)
            S_b = kp.tile([128, HS], f32)
            nc.vector.tensor_scalar(out=S_b[:], in0=colmax[:], scalar1=C127,
                                    scalar2=float(EPS), op0=A_.mult, op1=A_.max)
            R_b = kp.tile([128, HS], f32)
            nc.vector.reciprocal(R_b[:], S_b[:])
            # dequant per chunk: DVE mult, round (DVE/ACT alternating to
            # balance engines), DVE mult->bf16
            wq_sb = kp.tile([128, DCH, HS], bf16)
            for c in range(DCH):
                t1 = sb.tile([128, HS], f32, tag="t1")
                nc.vector.tensor_tensor(out=t1[:], in0=k_sb[:, c, :],
                                        in1=R_b[:], op=A_.mult)
                t3 = sb.tile([128, HS], f32, tag="t3")
                if c % 2 == 0:
                    nc.vector.tensor_scalar(out=t3[:], in0=t1[:],
                                            scalar1=MAGIC, scalar2=MAGIC,
                                            op0=A_.add, op1=A_.subtract)
                else:
                    t2 = sb.tile([128, HS], f32, tag="t2")
                    nc.scalar.activation(t2[:], t1[:], AF.Copy, bias=MAGIC)
                    nc.scalar.activation(t3[:], t2[:], AF.Copy, bias=-MAGIC)
                nc.vector.tensor_tensor(out=wq_sb[:, c, :], in0=t3[:],
                                        in1=S_b[:], op=A_.mult)
                if c % 2 == 1:
                    c2 = c // 2
                    ring = nc.sync if c2 % 2 == 0 else nc.scalar
                    ring.dma_start(
                        wq_o[c2 * 256:(c2 + 1) * 256, :].rearrange(
                            "(c p) h -> p c h", c=2),
                        wq_sb[:, 2 * c2:2 * c2 + 2, :])
    nc.compile()
    return nc


def _build_prog_b(loop_n=None):
    """Launch B: fused input quantize + row-parallel bf16 matmul.

    Blocked (t-half, H-half) in order (0,0),(1,0),(0,1),(1,1) with the wq
    halves in a 3-slot rotating pool and qiT/si double-buffered, so in the
    For_i timing loop iteration n+1's loads overlap iteration n's matmuls.
    Output is written bf16 (adds ~2e-3 rel err, halves output traffic);
    the host casts back to f32.
    """
    nc = bacc.Bacc("TRN2", target_bir_lowering=False, debug=False)
    x_dram = nc.dram_tensor("xa", [TR, D], f32, kind="ExternalInput")
    wq_dram = nc.dram_tensor("wqf", [D, H], bf16, kind="ExternalInput")
    out_o = nc.dram_tensor("out", [TR, H], bf16, kind="ExternalOutput")

    with tile.TileContext(nc) as tc:
        import contextlib
        with (
            tc.tile_pool(name="cst", bufs=1) as cst,
            tc.tile_pool(name="wqp", bufs=3) as wqp,
            tc.tile_pool(name="qtp", bufs=2) as qtp,
            tc.tile_pool(name="sip", bufs=2) as sip,
            tc.tile_pool(name="xp", bufs=8) as xp,
            tc.tile_pool(name="qp", bufs=8) as qp,
            tc.tile_pool(name="stg", bufs=2) as stg,
            tc.tile_pool(name="ssb", bufs=3) as ssb,
            tc.tile_pool(name="obp", bufs=8) as obp,
            tc.tile_pool(name="pp", bufs=4, space="PSUM") as pp,
            tc.tile_pool(name="tps", bufs=3, space="PSUM") as tps,
        ):
            ident = cst.tile([128, 128], bf16)
            make_identity(nc, ident[:])
            with (tc.For_i(0, loop_n, 1) if loop_n
                  else contextlib.nullcontext()):
                si_all = sip.tile([128, TT], f32, tag="si")
                qiT = qtp.tile([128, DCH, TR], bf16, tag="qiT")
                wq_h0 = wqp.tile([128, DCH, 2048], bf16, tag="wqh")
                wq_h1 = wqp.tile([128, DCH, 2048], bf16, tag="wqh")
                wq_h = [wq_h0, wq_h1]

                x_tiles = [None] * TT
                q_tiles = [None] * TT

                def load_x(t, ring):
                    x_sb = xp.tile([128, D], f32, tag="x")
                    ring.dma_start(x_sb[:], x_dram[t * 128:(t + 1) * 128, :])
                    x_tiles[t] = x_sb

                def quantize(t):
                    """DVE: row scales; round passes on ACT (tile 0 on DVE --
                    it gates the very first matmul group)."""
                    x_sb = x_tiles[t]
                    rmax = ssb.tile([128, 1], f32, tag="rmax")
                    nc.vector.tensor_reduce(rmax[:], x_sb[:], axis=AX.X,
                                            op=A_.max,
                                            apply_absolute_value=True)
                    nc.vector.tensor_scalar(out=si_all[:, t:t + 1],
                                            in0=rmax[:],
                                            scalar1=C127, scalar2=float(EPS),
                                            op0=A_.mult, op1=A_.max)
                    rr = ssb.tile([128, 1], f32, tag="rr")
                    nc.vector.reciprocal(rr[:], si_all[:, t:t + 1])
                    t_sb = stg.tile([128, D], f32, tag="t")
                    q_sb = qp.tile([128, D], bf16, tag="q")
                    if t <= 1:
                        nc.vector.tensor_scalar(out=t_sb[:], in0=x_sb[:],
                                                scalar1=rr[:], scalar2=MAGIC,
                                                op0=A_.mult, op1=A_.add)
                        nc.vector.tensor_scalar_sub(q_sb[:], t_sb[:], MAGIC)
                    else:
                        nc.scalar.activation(t_sb[:], x_sb[:], AF.Copy,
                                             bias=MAGIC, scale=rr[:])
                        nc.scalar.activation(q_sb[:], t_sb[:], AF.Copy,
                                             bias=-MAGIC, scale=1.0)
                    q_tiles[t] = q_sb

                def load_wq(qq):
                    hh, sub = qq // 2, qq % 2
                    for c in range(DCH):
                        nc.gpsimd.dma_start(
                            wq_h[hh][:, c, sub * 1024:(sub + 1) * 1024],
                            wq_dram[c * 128:(c + 1) * 128,
                                    qq * 1024:(qq + 1) * 1024])

                load_x(0, nc.gpsimd)
                for t in range(1, 4):
                    load_x(t, nc.sync)
                load_wq(0)
                load_wq(1)
                for t in range(4, TT):
                    load_x(t, nc.gpsimd)
                load_wq(2)
                load_wq(3)
                for t in range(4):
                    quantize(t)

                def transpose_quad(t, c0):
                    """PE-transpose chunks c0..c0+3 of q tile t into one psum
                    tile; a single DVE copy moves all four to qiT."""
                    pt = tps.tile([128, 4, 128], bf16, tag="pt")
                    for j in range(4):
                        c = c0 + j
                        nc.tensor.transpose(
                            pt[:, j, :],
                            q_tiles[t][:, c * 128:(c + 1) * 128],
                            ident[:])
                    nc.vector.tensor_copy(
                        out=qiT[:, c0:c0 + 4, t * 128:(t + 1) * 128],
                        in_=pt[:])

                for t in (0, 1):
                    for c0 in (0, 4):
                        transpose_quad(t, c0)

                # every quad must precede the first group reading its
                # tile; block 0's groups are ordered to chase the wq piece
                # arrival (qq0 columns first) while giving the tile 2-3
                # transposes later deadlines.
                t_sched = {
                    0: {4: [(3, 0)], 5: [(3, 4)],
                        10: [(4, 0)], 11: [(4, 4)], 12: [(5, 0)],
                        13: [(5, 4)], 14: [(6, 0)]},
                    1: {0: [(6, 4), (7, 0)], 1: [(7, 4)]},
                }
                order0 = [(0, 2), (1, 2), (0, 3), (1, 3),
                          (2, 0), (2, 1), (3, 0), (3, 1),
                          (2, 2), (2, 3), (3, 2), (3, 3)]
                dflt = [(j, tt_) for j in range(4) for tt_ in range(4)]

                o_open = {}

                def epilogue(i, t, ps):
                    last_blk = False
                    if i % 2 == 0:
                        o_sb = obp.tile([128, 1024], bf16, tag="o")
                        o_open[t] = (o_sb, i * 512)
                        nc.vector.tensor_scalar_mul(
                            o_sb[:, 0:512], ps[:], si_all[:, t:t + 1])
                    else:
                        o_sb, col0 = o_open.pop(t)
                        nc.scalar.activation(
                            o_sb[:, 512:1024], ps[:], AF.Copy,
                            scale=si_all[:, t:t + 1])
                        nc.sync.dma_start(
                            out_o[t * 128:(t + 1) * 128,
                                  col0:col0 + 1024], o_sb[:])

                # fused head: first four groups of block 0 run c-major so
                # each arriving wq piece feeds 4 accumulation groups at once
                head_groups = [(0, 0), (0, 1), (1, 0), (1, 1)]
                head_ps = []
                for hk in range(4):
                    ps = pp.tile([128, 512], f32, tag="ps")
                    head_ps.append(ps)
                for c in range(DCH):
                    for hk, (j, tt_) in enumerate(head_groups):
                        nc.tensor.matmul(
                            head_ps[hk][:],
                            qiT[:, c, tt_ * 128:(tt_ + 1) * 128],
                            wq_h[0][:, c, j * 512:(j + 1) * 512],
                            start=(c == 0), stop=(c == DCH - 1))
                    if c == 3:
                        transpose_quad(2, 0)
                    elif c == 5:
                        transpose_quad(2, 4)
                for hk, (j, tt_) in enumerate(head_groups):
                    epilogue(j, tt_, head_ps[hk])

                for bi, (th, ih) in enumerate(((0, 0), (1, 0),
                                              (0, 1), (1, 1))):
                    sched = t_sched.get(bi, {})
                    g = 4 if bi == 0 else 0
                    for (j, tt_) in (order0 if bi == 0 else dflt):
                        i = ih * 4 + j
                        col_l = j * 512
                        col = i * 512
                        if True:
                            t = th * 4 + tt_
                            ps = pp.tile([128, 512], f32, tag="ps")
                            for c in range(DCH):
                                nc.tensor.matmul(
                                    ps[:], qiT[:, c, t * 128:(t + 1) * 128],
                                    wq_h[ih][:, c, col_l:col_l + 512],
                                    start=(c == 0), stop=(c == DCH - 1))
                            for job in sched.get(g, ()):
                                transpose_quad(*job)
                            g += 1
                            if bi == 0 and g == 8:
                                for tq in range(4, TT):
                                    quantize(tq)
                            if bi == 3 and j >= 2:
                                # singles, all on ACT: DVE's stream ends
                                # early so the next loop iteration's
                                # quantize can overlap this tail
                                o_sb = obp.tile([128, 1024], bf16, tag="o")
                                nc.scalar.activation(
                                    o_sb[:, 0:512], ps[:], AF.Copy,
                                    scale=si_all[:, t:t + 1])
                                nc.sync.dma_start(
                                    out_o[t * 128:(t + 1) * 128,
                                          col:col + 512], o_sb[:, 0:512])
                            else:
                                epilogue(i, t, ps)
    nc.compile()
    return nc


# ---------------------------------------------------------------------------
# Runner: replicate bass2jax.run_bass_via_pjrt but cache the jitted callable.
# ---------------------------------------------------------------------------
class _Prog:
    def __init__(self, nc, n_cores=NCORES):
        import jax
        from jax.sharding import Mesh, PartitionSpec
        try:
            from jax.experimental.shard_map import shard_map
        except ImportError:
            from jax.shard_map import shard_map

        install_neuronx_cc_hook()
        self.nc = nc
        self.n_cores = n_cores
        partition_name = (nc.partition_id_tensor.name
                          if nc.partition_id_tensor else None)
        in_names, out_names, out_avals, zero_shapes = [], [], [], []
        for alloc in nc.m.functions[0].allocations:
            if not isinstance(alloc, mybir.MemoryLocationSet):
                continue
            name = alloc.memorylocations[0].name
            if alloc.kind == "ExternalInput":
                if name == partition_name:
                    continue
                in_names.append(name)
            elif alloc.kind == "ExternalOutput":
                out_names.append(name)
                shape = tuple(alloc.tensor_shape)
                dtype = mybir.dt.np(alloc.dtype)
                out_avals.append(jax.core.ShapedArray(shape, dtype))
                zero_shapes.append((shape, dtype))
        self.in_names = list(in_names)
        self.out_names = out_names
        self.out_avals = out_avals
        self.zero_shapes = zero_shapes
        n_params = len(in_names)
        n_outs = len(out_names)
        all_names = in_names + out_names
        if partition_name is not None:
            all_names = all_names + [partition_name]

        def _body(*args):
            operands = list(args)
            if partition_name is not None:
                operands.append(partition_id_tensor())
            outs = _bass_exec_p.bind(
                *operands,
                out_avals=tuple(out_avals),
                in_names=tuple(all_names),
                out_names=tuple(out_names),
                lowering_input_output_aliases=(),
                sim_require_finite=True,
                sim_require_nnan=True,
                nc=nc,
            )
            return tuple(outs)

        donate = tuple(range(n_params, n_params + n_outs))
        devices = jax.devices()[:n_cores]
        mesh = Mesh(np.asarray(devices), ("core",))
        self.mesh = mesh
        self.PartitionSpec = PartitionSpec
        self.n_params = n_params
        self.n_outs = n_outs
        in_specs = (PartitionSpec("core"),) * (n_params + n_outs)
        out_specs = (PartitionSpec("core"),) * n_outs
        self._body = _body
        self._shard_map = shard_map
        self.fn = jax.jit(
            shard_map(_body, mesh=mesh, in_specs=in_specs,
                      out_specs=out_specs, check_rep=False),
            donate_argnums=donate, keep_unused=True)
        self._chained = {}

    def chained_fn(self, n):
        """jit fn executing the NEFF n times sequentially (for timing)."""
        import jax

        if n in self._chained:
            return self._chained[n]

        def _body_n(*args):
            outs = None
            for _ in range(n):
                outs = self._body(*args)
            return outs

        in_specs = (self.PartitionSpec("core"),) * (self.n_params + self.n_outs)
        out_specs = (self.PartitionSpec("core"),) * self.n_outs
        fn = jax.jit(
            self._shard_map(_body_n, mesh=self.mesh, in_specs=in_specs,
                            out_specs=out_specs, check_rep=False),
            keep_unused=True)
        self._chained[n] = fn
        return fn

    def device_inputs(self, concat_in):
        """device_put inputs with the mesh sharding (axis 0 split)."""
        import jax
        from jax.sharding import NamedSharding

        sharding = NamedSharding(self.mesh, self.PartitionSpec("core"))
        out = [jax.device_put(a, sharding) for a in concat_in]
        for a in out:
            a.block_until_ready()
        return out

    def concat_inputs(self, in_maps):
        return [
            np.concatenate([np.asarray(m[name]) for m in in_maps], axis=0)
            for name in self.in_names
        ]

    def fresh_zeros(self):
        return [np.zeros((self.n_cores * s[0], *s[1:]), d)
                for (s, d) in self.zero_shapes]

    def run(self, concat_in):
        out_arrs = self.fn(*concat_in, *self.fresh_zeros())
        return out_arrs

    def split(self, out_arrs):
        res = []
        for c in range(self.n_cores):
            res.append({
                name: np.asarray(out_arrs[i]).reshape(
                    self.n_cores, *self.out_avals[i].shape)[c]
                for i, name in enumerate(self.out_names)
            })
        return res


def time_device(build_fn, concat_in_np, n_lo=8, n_hi=136, iters=4, reps=5):
    """Measure per-execution device time of a program by building loop_n
    variants (hardware For_i around the body) and differencing one-dispatch
    wall times. RPC/dispatch overhead (~90 ms) cancels in the delta.
    Repeats the paired measurement `reps` times and takes the min delta
    (occasional dispatches are inflated by ~10us of runtime noise)."""
    import time as _time

    progs = {}
    for n in (n_lo, n_hi):
        p = _Prog(build_fn(loop_n=n))
        fn = p.chained_fn(1)  # non-donating single-dispatch callable
        cin = p.device_inputs(concat_in_np)
        zeros = p.device_inputs(p.fresh_zeros())
        outs = fn(*cin, *zeros)
        outs[-1].block_until_ready()
        progs[n] = (fn, cin, zeros)
    deltas = []
    for _ in range(reps):
        times = {}
        for n in (n_lo, n_hi):
            fn, cin, zeros = progs[n]
            ts = []
            for _ in range(iters):
                t0 = _time.perf_counter()
                outs = fn(*cin, *zeros)
                outs[-1].block_until_ready()
                ts.append(_time.perf_counter() - t0)
            times[n] = min(ts)
        deltas.append((times[n_hi] - times[n_lo]) / (n_hi - n_lo))
    deltas.sort()
    return deltas[len(deltas) // 2]


_progs = {}


def _get_progs():
    if "a" not in _progs:
        _progs["a"] = _Prog(_build_prog_a2())
        _progs["b"] = _Prog(_build_prog_b())
    return _progs["a"], _progs["b"]


def make_in_maps_a(x, w):
    return [{"ka": w[:, c * HS:(c + 1) * HS]} for c in range(NCORES)]


def make_in_maps_b(x, res_a):
    wq_full = np.concatenate([r["wq"] for r in res_a], axis=1)   # [D, H] bf16
    return [
        {"xa": x[c * TR:(c + 1) * TR], "wqf": wq_full}
        for c in range(NCORES)
    ]


def kernel(inputs: np.ndarray, kernel: np.ndarray) -> np.ndarray:
    pa, pb = _get_progs()
    x = np.ascontiguousarray(np.asarray(inputs, dtype=np.float32).reshape(BT, D))
    w = np.ascontiguousarray(np.asarray(kernel, dtype=np.float32))

    res_a = pa.split(pa.run(pa.concat_inputs(make_in_maps_a(x, w))))
    res_b = pb.split(pb.run(pb.concat_inputs(make_in_maps_b(x, res_a))))

    out = np.concatenate([r["out"] for r in res_b], axis=0)         # [BT, H]
    return out.reshape(B, T, H).astype(np.float32)

